# revision 3
# baseline (speedup 1.0000x reference)
"""GCN (2x GCNConv + mean-pool + linear) on 8 Trainium2 NeuronCores.

Single-launch design: one bass kernel does
  AllGather(x shard) -> layer1 (gather/one-hot/matmul/relu) -> h1 shard
  -> AllGather(h1) -> layer2 + mean-pool + final linear -> per-core
  partial [G, OUT] summed on host.

Message gathers run in dma_gather transpose mode (measured ~2x cheaper
per descriptor than non-transpose); each 128-edge chunk is then PE-
transposed back to edge-major and PSUM-copied to SBUF before the
scatter-add one-hot matmuls (same algebra as the 2-launch baseline:
dst-side dinsq commutes out of relu; self-loop is an identity matmul
on the core's own rows; mean-pool folds into a valued indicator
matmul).
"""

import sys
from contextlib import ExitStack

for _p in ("/opt/trn_rl_repo",):
    if _p not in sys.path:
        sys.path.insert(0, _p)

import numpy as np

import concourse.bass as bass
import concourse.mybir as mybir
import concourse.tile as tile
from concourse import bacc
from concourse.bass_utils import run_bass_kernel_spmd
from concourse.library_config import mlp

FP16 = mybir.dt.float16
F32 = mybir.dt.float32
I16 = mybir.dt.int16
FP16_NP = np.float16
TGATHER = False
DEBUG_H1 = False


class Cfg:
    def __init__(self, N=100000, E=1600000, G=100, DIN=3, H=128, OUT=10,
                 NCORES=8, WT=8, SC_SIZE=25000):
        self.N, self.E, self.G = N, E, G
        self.DIN, self.H, self.OUT = DIN, H, OUT
        self.NCORES = NCORES
        assert N % NCORES == 0
        self.NPC = N // NCORES                      # nodes per core
        self.NT = (self.NPC + 127) // 128           # dst tiles per core
        self.LAST_VALID = self.NPC - (self.NT - 1) * 128
        self.WT = WT                                # tiles per wave
        self.NW = (self.NT + WT - 1) // WT
        assert SC_SIZE <= 32768
        self.SC_SIZE = SC_SIZE                      # src chunk rows (int16 idx)
        self.NSC = (N + SC_SIZE - 1) // SC_SIZE


FULL = Cfg()


# --------------------------------------------------------------------------
# host preprocessing (graph structure -> slot/window layout)
# --------------------------------------------------------------------------

def preprocess(cfg, x, edge_index, batch):
    N, G, NC = cfg.N, cfg.G, cfg.NCORES
    NPC, NT, NSC, WT = cfg.NPC, cfg.NT, cfg.NSC, cfg.WT
    src = np.asarray(edge_index[0], dtype=np.int64)
    dst = np.asarray(edge_index[1], dtype=np.int64)
    batch = np.asarray(batch, dtype=np.int64)
    x = np.asarray(x, dtype=np.float32)

    deg = (np.bincount(dst, minlength=N) + 1.0).astype(np.float32)
    dinsq = (1.0 / np.sqrt(deg)).astype(np.float32)
    invdeg = (dinsq * dinsq).astype(np.float32)
    sqrtdeg = np.sqrt(deg).astype(np.float32)
    cnt = np.bincount(batch, minlength=G).astype(np.float32)
    invcnt = (1.0 / np.maximum(cnt, 1.0)).astype(np.float32)

    # gather table for layer 1: fp16 x rows scaled by dinsq, padded to 128
    x_pad = np.zeros((N, 128), dtype=FP16_NP)
    x_pad[:, :cfg.DIN] = (x * dinsq[:, None]).astype(FP16_NP)

    core = dst // NPC
    dst_local = dst - core * NPC
    tl = dst_local >> 7
    sc = src // cfg.SC_SIZE
    key = (core * NT + tl) * NSC + sc
    order = np.argsort(key, kind="stable")
    key_s = key[order]
    src_s = src[order]
    sc_s = sc[order]

    counts = np.bincount(key_s, minlength=NC * NT * NSC).reshape(NC, NT, NSC)
    # per-(tile, srcchunk) slot count, padded to 32 and uniform across cores
    GRAIN = 32
    P = ((counts.max(axis=0) + GRAIN - 1) // GRAIN * GRAIN).astype(np.int64)

    waves = [list(range(w * WT, min((w + 1) * WT, NT))) for w in range(cfg.NW)]
    slot_base = np.zeros((NT, NSC), dtype=np.int64)   # global slot index
    gmeta = []     # per wave: per s: (idx_col0, nidx, msgcol0, nch)
    wmms = []      # per wave: ordered list of (mcol, j_in_wave, wid, gcol)
    pos = 0        # global chunk counter
    SENT = 16384.0
    for w, wtiles in enumerate(waves):
        wmeta = []
        wave_chunk0 = pos
        mms = []
        for s in range(NSC):
            c0 = pos
            off = 0
            spans = []  # (t, slot_lo, slot_hi) within group
            for t in wtiles:
                slot_base[t, s] = c0 * 128 + off
                if P[t, s]:
                    spans.append((t, off, off + int(P[t, s])))
                off += int(P[t, s])
            nch = (off + 127) // 128
            for k in range(nch):
                lo, hi = k * 128, (k + 1) * 128
                sp = [t for t, a, b in spans if a < hi and b > lo]
                if not sp:
                    continue
                j0, j1 = sp[0] - wtiles[0], sp[-1] - wtiles[0]
                jj = j0
                while jj <= j1:          # split windows at psum-bank groups
                    je = min(j1, (jj // 4) * 4 + 3)
                    mms.append((c0 - wave_chunk0 + k, jj, je - jj + 1, c0 + k))
                    jj = je + 1
            pos += nch
            wmeta.append((c0 * 8, nch * 128, c0 - wave_chunk0, nch))
        gmeta.append(wmeta)
        wmms.append(mms)
    TOTCH = pos
    CW = max(sum(gmeta[w][s][3] for s in range(NSC)) for w in range(cfg.NW))

    # scatter edge data into padded per-core arrays
    idx_all = np.zeros((NC, TOTCH * 128), dtype=np.int16)
    dstl_all = np.full((NC, TOTCH * 128), SENT, dtype=np.float32)
    bstart = np.zeros(NC * NT * NSC, dtype=np.int64)
    cflat = counts.reshape(-1)
    bstart[1:] = np.cumsum(cflat)[:-1]
    rank = np.arange(len(key_s)) - bstart[key_s]
    ccore = key_s // (NT * NSC)
    rem = key_s % (NT * NSC)
    dest = slot_base.reshape(-1)[rem] + rank
    idx_all[ccore, dest] = (src_s - sc_s * cfg.SC_SIZE).astype(np.int16)
    wavebase = (tl[order] // WT) * WT * 128
    dstl_all[ccore, dest] = (dst_local[order] - wavebase).astype(np.float32)

    # compact idx: [16, TOTCH*8] per core (device replicates to 128 rows)
    idx16 = np.ascontiguousarray(
        idx_all.reshape(NC, TOTCH * 8, 16).transpose(0, 2, 1))
    dstl_wrap = np.ascontiguousarray(
        dstl_all.reshape(NC, TOTCH, 128).transpose(0, 2, 1))

    # per-core per-node columns (padded to NT*128)
    NPAD = NT * 128
    invdeg_col = np.zeros((NC, 128, NT), dtype=np.float32)
    sqrtdeg_row = np.ones((NC, 1, NPAD), dtype=np.float32)
    pbat_col = np.zeros((NC, 128, NT), dtype=np.float32)
    pwv_col = np.zeros((NC, 128, NT), dtype=np.float32)
    for c in range(NC):
        idx = np.arange(NPAD) + c * NPC
        valid = np.arange(NPAD) < NPC
        idx = np.where(valid, idx, 0)
        iv = np.where(valid, invdeg[idx], 1.0).astype(np.float32)
        invdeg_col[c] = iv.reshape(NT, 128).T
        sqrtdeg_row[c, 0] = np.where(valid, sqrtdeg[idx], 1.0)
        pb = np.where(valid, batch[idx].astype(np.float32), -1.0)
        pbat_col[c] = pb.reshape(NT, 128).T
        wv = np.where(valid, dinsq[idx] * invcnt[batch[idx]], 0.0)
        pwv_col[c] = wv.astype(np.float32).reshape(NT, 128).T

    iota = np.tile(np.arange(WT * 128, dtype=np.float32),
                   (128, 1)).astype(FP16_NP)
    iotag = np.tile(np.arange(G, dtype=np.float32), (128, 1)).astype(FP16_NP)
    ident = np.eye(128, dtype=np.float32)

    return dict(
        x_pad=x_pad, idx16=idx16, dstl_wrap=dstl_wrap,
        invdeg_col=invdeg_col, sqrtdeg_row=sqrtdeg_row,
        pbat_col=pbat_col, pwv_col=pwv_col,
        iota=iota, iotag=iotag, ident=ident,
        waves=waves, gmeta=gmeta, wmms=wmms,
        TOTCH=TOTCH, CW=CW, deg=deg,
    )


# --------------------------------------------------------------------------
# the fused two-layer kernel
# --------------------------------------------------------------------------

def build_fused(cfg, meta, has_b1, has_b2):
    N, G, OUT = cfg.N, cfg.G, cfg.OUT
    NT, NSC, WT, NPC, NC = cfg.NT, cfg.NSC, cfg.WT, cfg.NPC, cfg.NCORES
    TOTCH, CW = meta["TOTCH"], meta["CW"]
    waves, gmeta, wmms = meta["waves"], meta["gmeta"], meta["wmms"]
    NPAD = NT * 128

    nc = bacc.Bacc("TRN2", target_bir_lowering=False, debug=False,
                   num_swdge_queues=4, dynamic_dma_scratch_size=32768,
                   num_devices=NC)
    xsh_d = nc.dram_tensor("xsh", [NPC, 128], FP16, kind="ExternalInput")
    idx_d = nc.dram_tensor("idx", [16, TOTCH * 8], I16, kind="ExternalInput")
    dstl_d = nc.dram_tensor("dstl", [128, TOTCH], F32, kind="ExternalInput")
    iota_d = nc.dram_tensor("iota", [128, WT * 128], FP16, kind="ExternalInput")
    iotag_d = nc.dram_tensor("iotag", [128, G], FP16, kind="ExternalInput")
    ident_d = nc.dram_tensor("ident", [128, 128], F32, kind="ExternalInput")
    w1_d = nc.dram_tensor("w1", [4, 128], F32, kind="ExternalInput")
    w2_d = nc.dram_tensor("w2", [128, 128], F32, kind="ExternalInput")
    wl_d = nc.dram_tensor("wl", [128, OUT], F32, kind="ExternalInput")
    ivd_d = nc.dram_tensor("ivd", [128, NT], F32, kind="ExternalInput")
    pbat_d = nc.dram_tensor("pbat", [128, NT], F32, kind="ExternalInput")
    pwv_d = nc.dram_tensor("pwv", [128, NT], F32, kind="ExternalInput")
    if has_b1 or has_b2:
        sqd_d = nc.dram_tensor("sqd", [1, NPAD], F32, kind="ExternalInput")
    if has_b1:
        b1_d = nc.dram_tensor("b1r", [1, 128], F32, kind="ExternalInput")
    if has_b2:
        b2_d = nc.dram_tensor("b2r", [1, 128], F32, kind="ExternalInput")
    out_d = nc.dram_tensor("out", [G, OUT], F32, kind="ExternalOutput")
    if DEBUG_H1:
        h1dump_d = nc.dram_tensor("h1dump", [NPC, 128], FP16,
                                  kind="ExternalOutput")

    relu = mybir.ActivationFunctionType.Relu
    copyf = mybir.ActivationFunctionType.Copy

    with tile.TileContext(nc) as tc:
        nc.gpsimd.load_library(mlp)
        with ExitStack() as ctx:
            const = ctx.enter_context(tc.tile_pool(name="const", bufs=1))
            dram = ctx.enter_context(tc.tile_pool(name="dram", bufs=1,
                                                  space="DRAM"))
            mtp = ctx.enter_context(tc.tile_pool(name="mt", bufs=2))
            msgp = ctx.enter_context(tc.tile_pool(name="msg", bufs=2))
            ohp = ctx.enter_context(tc.tile_pool(name="oh", bufs=4))
            asbp = ctx.enter_context(tc.tile_pool(name="asb", bufs=2))
            rlp = ctx.enter_context(tc.tile_pool(name="rl", bufs=2))
            stp = ctx.enter_context(tc.tile_pool(name="st", bufs=2))
            ownp = ctx.enter_context(tc.tile_pool(name="own", bufs=3))
            pwp = ctx.enter_context(tc.tile_pool(name="pw", bufs=3))
            aggp = ctx.enter_context(tc.tile_pool(name="agg", bufs=2,
                                                  space="PSUM"))
            p2p = ctx.enter_context(tc.tile_pool(name="p2", bufs=1,
                                                 space="PSUM"))
            mtrp = ctx.enter_context(tc.tile_pool(name="mtr", bufs=2,
                                                  space="PSUM"))
            trp = ctx.enter_context(tc.tile_pool(name="tr", bufs=2,
                                                 space="PSUM"))
            plp = ctx.enter_context(tc.tile_pool(name="pl", bufs=1,
                                                 space="PSUM"))

            # ---- DRAM scratch + collectives --------------------------------
            xloc = dram.tile([NPC, 128], FP16)
            tabx = dram.tile([N, 128], FP16)
            h1loc = dram.tile([NPC, 128], FP16)
            tabh = dram.tile([N, 128], FP16)

            nc.sync.dma_start(xloc[:], xsh_d[:])
            nc.gpsimd.collective_compute(
                "AllGather", mybir.AluOpType.bypass,
                replica_groups=[list(range(NC))],
                ins=[xloc.opt()], outs=[tabx.opt()])

            # ---- constants -------------------------------------------------
            idx_t = const.tile([128, TOTCH * 8], I16)
            for r in range(8):
                nc.sync.dma_start(idx_t[r * 16:(r + 1) * 16, :], idx_d[:])
            dstl_t = const.tile([128, TOTCH], F32)
            nc.sync.dma_start(dstl_t[:], dstl_d[:])
            iota_t = const.tile([128, WT * 128], FP16)
            nc.sync.dma_start(iota_t[:], iota_d[:])
            iotag_t = const.tile([128, G], FP16)
            nc.sync.dma_start(iotag_t[:], iotag_d[:])
            ident_t = const.tile([128, 128], F32)
            nc.sync.dma_start(ident_t[:], ident_d[:])
            identh_t = const.tile([128, 128], FP16)
            nc.any.tensor_copy(identh_t[:], ident_t[:])
            zc_t = const.tile([1, 512], FP16)
            nc.vector.memset(zc_t[:], 0.0)
            w1_t = const.tile([4, 128], F32)
            nc.sync.dma_start(w1_t[:], w1_d[:])
            w2_t = const.tile([128, 128], F32)
            nc.sync.dma_start(w2_t[:], w2_d[:])
            wl_t = const.tile([128, OUT], F32)
            nc.sync.dma_start(wl_t[:], wl_d[:])
            ivd_t = const.tile([128, NT], F32)
            nc.sync.dma_start(ivd_t[:], ivd_d[:])
            pbat_t = const.tile([128, NT], F32)
            nc.sync.dma_start(pbat_t[:], pbat_d[:])
            pwv_t = const.tile([128, NT], F32)
            nc.sync.dma_start(pwv_t[:], pwv_d[:])
            if has_b1 or has_b2:
                sqd_t = const.tile([1, NPAD], F32)
                nc.sync.dma_start(sqd_t[:], sqd_d[:])
            if has_b1:
                b1_t = const.tile([1, 128], F32)
                nc.sync.dma_start(b1_t[:], b1_d[:])
            if has_b2:
                b2_t = const.tile([1, 128], F32)
                nc.sync.dma_start(b2_t[:], b2_d[:])

            pooled_ps = plp.tile([128, G], F32)

            CALL_CHUNKS = 48

            def layer(lnum):
                KIN = 4 if lnum == 1 else 128
                tab = tabx if lnum == 1 else tabh
                own_src = xsh_d if lnum == 1 else h1loc
                has_b = has_b1 if lnum == 1 else has_b2
                brow = (b1_t if lnum == 1 else b2_t) if has_b else None
                w_t = w1_t if lnum == 1 else w2_t
                gq = 0
                for w, wtiles in enumerate(waves):
                    msg = msgp.tile([128, CW, KIN if TGATHER else 128],
                                    FP16, tag="msg")
                    mts = {}
                    for s in range(NSC):
                        icol0, nidx, mcol0, nch = gmeta[w][s]
                        if nidx == 0:
                            continue
                        r0 = s * cfg.SC_SIZE
                        r1 = min(N, r0 + cfg.SC_SIZE)
                        for cb in range(0, nch, CALL_CHUNKS):
                            ce = min(cb + CALL_CHUNKS, nch)
                            ni = (ce - cb) * 128
                            if TGATHER:
                                mt = mtp.tile([128, ni], FP16, tag="mt")
                                nc.gpsimd.dma_gather(
                                    mt[:].rearrange("p (c i) -> p c i", c=1),
                                    tab[r0:r1, :],
                                    idx_t[:, icol0 + cb * 8:
                                          icol0 + cb * 8 + ni // 16],
                                    ni, ni, 128,
                                    transpose=True,
                                    single_packet=False,
                                    queue_num=gq % 4,
                                )
                                for k in range(ce - cb):
                                    mts[mcol0 + cb + k] = (mt, k)
                            else:
                                nc.gpsimd.dma_gather(
                                    msg[:, mcol0 + cb:mcol0 + ce, :],
                                    tab[r0:r1, :],
                                    idx_t[:, icol0 + cb * 8:
                                          icol0 + cb * 8 + ni // 16],
                                    ni, ni, 128,
                                    single_packet=False,
                                    queue_num=gq % 4,
                                )
                            gq += 1
                    # per-chunk PE transpose back to edge-major + PSUM->SBUF
                    for mcol, (mt, k) in mts.items():
                        mtr = mtrp.tile([128, 128], FP16, tag="mtr")
                        nc.tensor.transpose(
                            mtr[:], mt[:, k * 128:(k + 1) * 128], identh_t[:])
                        nc.scalar.activation(msg[:, mcol, :], mtr[:, 0:KIN],
                                             copyf)
                    # psum scatter-add accumulation
                    mms = wmms[w]
                    aggs = [aggp.tile([KIN, 512], F32, tag="agg",
                                      name=f"agg{lnum}_w{w}_{h}")
                            for h in range((len(wtiles) + 3) // 4)]
                    for agg in aggs:
                        nc.tensor.matmul(agg[:], zc_t[0:1, 0:KIN],
                                         zc_t[0:1, 0:512],
                                         start=True, stop=False,
                                         skip_group_check=True)
                    for mcol, j0, wid, gcol in mms:
                        oh = ohp.tile([128, wid * 128], FP16, tag="oh")
                        nc.vector.tensor_scalar(
                            oh[:], iota_t[:, j0 * 128:(j0 + wid) * 128],
                            dstl_t[:, gcol:gcol + 1], None,
                            mybir.AluOpType.is_equal)
                        agg = aggs[j0 // 4]
                        psl = agg[:, (j0 % 4) * 128:(j0 % 4 + wid) * 128]
                        nc.tensor.matmul(
                            psl, msg[:, mcol, 0:KIN], oh[:],
                            start=False, stop=False, skip_group_check=True)
                    # self-loop term
                    for j, t in enumerate(wtiles):
                        rows = min(128, NPC - t * 128)
                        own_t = ownp.tile([128, 128], FP16, tag="own")
                        nc.sync.dma_start(own_t[0:rows, :],
                                          own_src[t * 128:t * 128 + rows, :])
                        psl = aggs[j // 4][:, (j % 4) * 128:(j % 4) * 128 + 128]
                        nc.tensor.matmul(
                            psl, own_t[0:rows, 0:KIN], identh_t[0:rows, :],
                            start=False,
                            stop=(j % 4 == 3 or j == len(wtiles) - 1),
                            skip_group_check=True)
                    # epilogue per tile
                    for j, t in enumerate(wtiles):
                        psl = aggs[j // 4][:, (j % 4) * 128:(j % 4) * 128 + 128]
                        agg_sb = asbp.tile([KIN, 128], F32, tag="asb")
                        nc.scalar.activation(agg_sb[:], psl, copyf)
                        p2 = p2p.tile([128, 128], F32, tag="p2")
                        nc.tensor.matmul(p2[:], w_t[:], agg_sb[:],
                                         start=True, stop=not has_b)
                        if has_b:
                            nc.tensor.matmul(p2[:], brow[:],
                                             sqd_t[0:1, t * 128:t * 128 + 128],
                                             start=False, stop=True)
                        relu_sb = rlp.tile([128, 128], F32, tag="rl")
                        nc.scalar.activation(relu_sb[:], p2[:], relu)
                        tnm = trp.tile([128, 128], F32, tag="tr")
                        nc.tensor.transpose(tnm[:], relu_sb[:], ident_t[:])
                        if lnum == 1:
                            if j == 0:
                                stage = stp.tile([128, WT * 128], FP16,
                                                 tag="stage")
                            nc.scalar.activation(
                                stage[:, j * 128:j * 128 + 128], tnm[:],
                                copyf, scale=ivd_t[:, t:t + 1])
                        else:
                            tnm_sb = stp.tile([128, 128], F32, tag="tnm")
                            nc.scalar.activation(tnm_sb[:], tnm[:], copyf)
                            pw_t = pwp.tile([128, G], F32, tag="pw")
                            nc.vector.tensor_scalar(
                                pw_t[:], iotag_t[:],
                                pbat_t[:, t:t + 1], None,
                                mybir.AluOpType.is_equal)
                            nc.vector.tensor_scalar(
                                pw_t[:], pw_t[:],
                                pwv_t[:, t:t + 1], None,
                                mybir.AluOpType.mult)
                            nc.tensor.matmul(pooled_ps[:], tnm_sb[:], pw_t[:],
                                             start=(t == 0), stop=(t == NT - 1),
                                             skip_group_check=True)
                    if lnum == 1:
                        # store wave's node-major rows (fp16) into h1loc
                        base = wtiles[0] * 128
                        nfull = sum(1 for t in wtiles if (t + 1) * 128 <= NPC)
                        if nfull:
                            dst_ap = h1loc[base:base + nfull * 128, :].rearrange(
                                "(j p) f -> p j f", p=128)
                            nc.sync.dma_start(
                                dst_ap, stage[:, 0:nfull * 128]
                                .rearrange("p (j f) -> p j f", f=128))
                        for j, t in enumerate(wtiles):
                            if (t + 1) * 128 <= NPC:
                                continue
                            rows = NPC - t * 128
                            if rows > 0:
                                nc.sync.dma_start(
                                    h1loc[t * 128:t * 128 + rows, :],
                                    stage[0:rows, j * 128:(j + 1) * 128])

            layer(1)
            if DEBUG_H1:
                nc.sync.dma_start(h1dump_d[:], h1loc[:])
            nc.gpsimd.collective_compute(
                "AllGather", mybir.AluOpType.bypass,
                replica_groups=[list(range(NC))],
                ins=[h1loc.opt()], outs=[tabh.opt()])
            layer(2)

            pooled_sb = const.tile([128, G], F32)
            nc.any.tensor_copy(pooled_sb[:], pooled_ps[:])
            outp = p2p.tile([128, 128], F32, tag="p2")
            nc.tensor.matmul(outp[0:G, 0:OUT], pooled_sb[:], wl_t[:],
                             start=True, stop=True, skip_group_check=True)
            out_sb = const.tile([G, OUT], F32)
            nc.any.tensor_copy(out_sb[:], outp[0:G, 0:OUT])
            nc.sync.dma_start(out_d[:], out_sb[:])

    nc.compile()
    return nc


# --------------------------------------------------------------------------
# driver
# --------------------------------------------------------------------------

def make_in_maps(cfg, meta, W1, b1, W2, b2, Wl, bl):
    NC, NPC = cfg.NCORES, cfg.NPC
    has_b1 = bool(np.any(np.asarray(b1)))
    has_b2 = bool(np.any(np.asarray(b2)))
    assert cfg.DIN <= 4
    W1p = np.zeros((4, 128), dtype=np.float32)
    W1p[:cfg.DIN] = np.asarray(W1, dtype=np.float32)
    in_maps = []
    for c in range(NC):
        m = dict(
            xsh=meta["x_pad"][c * NPC:(c + 1) * NPC],
            idx=meta["idx16"][c],
            dstl=meta["dstl_wrap"][c],
            iota=meta["iota"], iotag=meta["iotag"], ident=meta["ident"],
            w1=W1p, w2=np.asarray(W2, np.float32),
            wl=np.asarray(Wl, np.float32),
            ivd=meta["invdeg_col"][c],
            pbat=meta["pbat_col"][c], pwv=meta["pwv_col"][c],
        )
        if has_b1 or has_b2:
            m["sqd"] = meta["sqrtdeg_row"][c]
        if has_b1:
            m["b1r"] = np.asarray(b1, np.float32).reshape(1, 128)
        if has_b2:
            m["b2r"] = np.asarray(b2, np.float32).reshape(1, 128)
        in_maps.append(m)
    return in_maps, has_b1, has_b2


def _run(cfg, meta, W1, b1, W2, b2, Wl, bl, runner):
    in_maps, has_b1, has_b2 = make_in_maps(cfg, meta, W1, b1, W2, b2, Wl, bl)
    nc = build_fused(cfg, meta, has_b1, has_b2)
    res = runner(nc, in_maps)
    total = np.sum([res[c]["out"] for c in range(cfg.NCORES)], axis=0)
    return (total + np.asarray(bl, np.float32)[None, :]).astype(np.float32)


def _hw_runner(nc, in_maps):
    # one bounded retry: the axon/fake_nrt path occasionally throws a
    # transient "mesh desynced" on the first execute after device churn
    try:
        res = run_bass_kernel_spmd(nc, in_maps,
                                   core_ids=list(range(len(in_maps))))
    except Exception:
        import time as _time
        _time.sleep(10)
        res = run_bass_kernel_spmd(nc, in_maps,
                                   core_ids=list(range(len(in_maps))))
    return res.results


def kernel(x, edge_index, batch, W1, b1, W2, b2, Wl, bl):
    cfg = FULL
    meta = preprocess(cfg, x, edge_index, batch)
    return _run(cfg, meta, W1, b1, W2, b2, Wl, bl, _hw_runner)


# revision 4
# speedup vs baseline: 1.1227x; 1.1227x over previous
"""GCN (2x GCNConv + mean-pool + linear) on 8 Trainium2 NeuronCores.

Single-launch design: one bass kernel does
  AllGather(x shard) -> layer1 (gather/one-hot/matmul/relu) -> h1 shard
  -> AllGather(h1) -> layer2 + mean-pool + final linear -> per-core
  partial [G, OUT] summed on host.

Message gathers run in dma_gather transpose mode (measured ~2x cheaper
per descriptor than non-transpose); each 128-edge chunk is then PE-
transposed back to edge-major and PSUM-copied to SBUF before the
scatter-add one-hot matmuls (same algebra as the 2-launch baseline:
dst-side dinsq commutes out of relu; self-loop is an identity matmul
on the core's own rows; mean-pool folds into a valued indicator
matmul).
"""

import sys
from contextlib import ExitStack

for _p in ("/opt/trn_rl_repo",):
    if _p not in sys.path:
        sys.path.insert(0, _p)

import numpy as np

import concourse.bass as bass
import concourse.mybir as mybir
import concourse.tile as tile
from concourse import bacc
from concourse.bass_utils import run_bass_kernel_spmd
from concourse.library_config import mlp

FP16 = mybir.dt.float16
F32 = mybir.dt.float32
I16 = mybir.dt.int16
FP16_NP = np.float16
TGATHER = False
DEBUG_H1 = False


class Cfg:
    def __init__(self, N=100000, E=1600000, G=100, DIN=3, H=128, OUT=10,
                 NCORES=8, WT=8, SC_SIZE=25000):
        self.N, self.E, self.G = N, E, G
        self.DIN, self.H, self.OUT = DIN, H, OUT
        self.NCORES = NCORES
        assert N % NCORES == 0
        self.NPC = N // NCORES                      # nodes per core
        self.NT = (self.NPC + 127) // 128           # dst tiles per core
        self.LAST_VALID = self.NPC - (self.NT - 1) * 128
        self.WT = WT                                # tiles per wave
        self.NW = (self.NT + WT - 1) // WT
        assert SC_SIZE <= 32768
        self.SC_SIZE = SC_SIZE                      # src chunk rows (int16 idx)
        self.NSC = (N + SC_SIZE - 1) // SC_SIZE


FULL = Cfg()


# --------------------------------------------------------------------------
# host preprocessing (graph structure -> slot/window layout)
# --------------------------------------------------------------------------

def preprocess(cfg, x, edge_index, batch):
    N, G, NC = cfg.N, cfg.G, cfg.NCORES
    NPC, NT, NSC, WT = cfg.NPC, cfg.NT, cfg.NSC, cfg.WT
    src = np.asarray(edge_index[0], dtype=np.int64)
    dst = np.asarray(edge_index[1], dtype=np.int64)
    batch = np.asarray(batch, dtype=np.int64)
    x = np.asarray(x, dtype=np.float32)

    deg = (np.bincount(dst, minlength=N) + 1.0).astype(np.float32)
    dinsq = (1.0 / np.sqrt(deg)).astype(np.float32)
    invdeg = (dinsq * dinsq).astype(np.float32)
    sqrtdeg = np.sqrt(deg).astype(np.float32)
    cnt = np.bincount(batch, minlength=G).astype(np.float32)
    invcnt = (1.0 / np.maximum(cnt, 1.0)).astype(np.float32)

    # gather table for layer 1: fp16 x rows scaled by dinsq, padded to 128
    x_pad = np.zeros((N, 128), dtype=FP16_NP)
    x_pad[:, :cfg.DIN] = (x * dinsq[:, None]).astype(FP16_NP)

    core = dst // NPC
    dst_local = dst - core * NPC
    tl = dst_local >> 7
    sc = src // cfg.SC_SIZE
    key = (core * NT + tl) * NSC + sc
    order = np.argsort(key, kind="stable")
    key_s = key[order]
    src_s = src[order]
    sc_s = sc[order]

    counts = np.bincount(key_s, minlength=NC * NT * NSC).reshape(NC, NT, NSC)
    # per-(tile, srcchunk) slot count, padded to 32 and uniform across cores
    GRAIN = 1
    P = ((counts.max(axis=0) + GRAIN - 1) // GRAIN * GRAIN).astype(np.int64)

    waves = [list(range(w * WT, min((w + 1) * WT, NT))) for w in range(cfg.NW)]
    slot_base = np.zeros((NT, NSC), dtype=np.int64)   # global slot index
    gmeta = []     # per wave: per s: (idx_col0, nidx, msgcol0, nch)
    wmms = []      # per wave: ordered list of (mcol, j_in_wave, wid, gcol)
    pos = 0        # global chunk counter
    SENT = 16384.0
    for w, wtiles in enumerate(waves):
        wmeta = []
        wave_chunk0 = pos
        mms = []
        for s in range(NSC):
            c0 = pos
            off = 0
            spans = []  # (t, slot_lo, slot_hi) within group
            for t in wtiles:
                slot_base[t, s] = c0 * 128 + off
                if P[t, s]:
                    spans.append((t, off, off + int(P[t, s])))
                off += int(P[t, s])
            nch = (off + 127) // 128
            for k in range(nch):
                lo, hi = k * 128, (k + 1) * 128
                sp = [t for t, a, b in spans if a < hi and b > lo]
                if not sp:
                    continue
                j0, j1 = sp[0] - wtiles[0], sp[-1] - wtiles[0]
                jj = j0
                while jj <= j1:          # split windows at psum-bank groups
                    je = min(j1, (jj // 4) * 4 + 3)
                    mms.append((c0 - wave_chunk0 + k, jj, je - jj + 1, c0 + k))
                    jj = je + 1
            pos += nch
            wmeta.append((c0 * 8, nch * 128, c0 - wave_chunk0, nch))
        gmeta.append(wmeta)
        wmms.append(mms)
    TOTCH = pos
    CW = max(sum(gmeta[w][s][3] for s in range(NSC)) for w in range(cfg.NW))

    # scatter edge data into padded per-core arrays
    idx_all = np.zeros((NC, TOTCH * 128), dtype=np.int16)
    dstl_all = np.full((NC, TOTCH * 128), SENT, dtype=np.float32)
    bstart = np.zeros(NC * NT * NSC, dtype=np.int64)
    cflat = counts.reshape(-1)
    bstart[1:] = np.cumsum(cflat)[:-1]
    rank = np.arange(len(key_s)) - bstart[key_s]
    ccore = key_s // (NT * NSC)
    rem = key_s % (NT * NSC)
    dest = slot_base.reshape(-1)[rem] + rank
    idx_all[ccore, dest] = (src_s - sc_s * cfg.SC_SIZE).astype(np.int16)
    wavebase = (tl[order] // WT) * WT * 128
    dstl_all[ccore, dest] = (dst_local[order] - wavebase).astype(np.float32)

    # compact idx: [16, TOTCH*8] per core (device replicates to 128 rows)
    idx16 = np.ascontiguousarray(
        idx_all.reshape(NC, TOTCH * 8, 16).transpose(0, 2, 1))
    dstl_wrap = np.ascontiguousarray(
        dstl_all.reshape(NC, TOTCH, 128).transpose(0, 2, 1))

    # per-core per-node columns (padded to NT*128)
    NPAD = NT * 128
    invdeg_col = np.zeros((NC, 128, NT), dtype=np.float32)
    sqrtdeg_row = np.ones((NC, 1, NPAD), dtype=np.float32)
    pbat_col = np.zeros((NC, 128, NT), dtype=np.float32)
    pwv_col = np.zeros((NC, 128, NT), dtype=np.float32)
    for c in range(NC):
        idx = np.arange(NPAD) + c * NPC
        valid = np.arange(NPAD) < NPC
        idx = np.where(valid, idx, 0)
        iv = np.where(valid, invdeg[idx], 1.0).astype(np.float32)
        invdeg_col[c] = iv.reshape(NT, 128).T
        sqrtdeg_row[c, 0] = np.where(valid, sqrtdeg[idx], 1.0)
        pb = np.where(valid, batch[idx].astype(np.float32), -1.0)
        pbat_col[c] = pb.reshape(NT, 128).T
        wv = np.where(valid, dinsq[idx] * invcnt[batch[idx]], 0.0)
        pwv_col[c] = wv.astype(np.float32).reshape(NT, 128).T

    iota = np.tile(np.arange(WT * 128, dtype=np.float32),
                   (128, 1)).astype(FP16_NP)
    iotag = np.tile(np.arange(G, dtype=np.float32), (128, 1)).astype(FP16_NP)
    ident = np.eye(128, dtype=np.float32)

    return dict(
        x_pad=x_pad, idx16=idx16, dstl_wrap=dstl_wrap,
        invdeg_col=invdeg_col, sqrtdeg_row=sqrtdeg_row,
        pbat_col=pbat_col, pwv_col=pwv_col,
        iota=iota, iotag=iotag, ident=ident,
        waves=waves, gmeta=gmeta, wmms=wmms,
        TOTCH=TOTCH, CW=CW, deg=deg,
    )


# --------------------------------------------------------------------------
# the fused two-layer kernel
# --------------------------------------------------------------------------

def build_fused(cfg, meta, has_b1, has_b2):
    N, G, OUT = cfg.N, cfg.G, cfg.OUT
    NT, NSC, WT, NPC, NC = cfg.NT, cfg.NSC, cfg.WT, cfg.NPC, cfg.NCORES
    TOTCH, CW = meta["TOTCH"], meta["CW"]
    waves, gmeta, wmms = meta["waves"], meta["gmeta"], meta["wmms"]
    NPAD = NT * 128

    nc = bacc.Bacc("TRN2", target_bir_lowering=False, debug=False,
                   num_swdge_queues=4, dynamic_dma_scratch_size=32768,
                   num_devices=NC)
    xsh_d = nc.dram_tensor("xsh", [NPC, 128], FP16, kind="ExternalInput")
    idx_d = nc.dram_tensor("idx", [16, TOTCH * 8], I16, kind="ExternalInput")
    dstl_d = nc.dram_tensor("dstl", [128, TOTCH], F32, kind="ExternalInput")
    iota_d = nc.dram_tensor("iota", [128, WT * 128], FP16, kind="ExternalInput")
    iotag_d = nc.dram_tensor("iotag", [128, G], FP16, kind="ExternalInput")
    ident_d = nc.dram_tensor("ident", [128, 128], F32, kind="ExternalInput")
    w1_d = nc.dram_tensor("w1", [4, 128], F32, kind="ExternalInput")
    w2_d = nc.dram_tensor("w2", [128, 128], F32, kind="ExternalInput")
    wl_d = nc.dram_tensor("wl", [128, OUT], F32, kind="ExternalInput")
    ivd_d = nc.dram_tensor("ivd", [128, NT], F32, kind="ExternalInput")
    pbat_d = nc.dram_tensor("pbat", [128, NT], F32, kind="ExternalInput")
    pwv_d = nc.dram_tensor("pwv", [128, NT], F32, kind="ExternalInput")
    if has_b1 or has_b2:
        sqd_d = nc.dram_tensor("sqd", [1, NPAD], F32, kind="ExternalInput")
    if has_b1:
        b1_d = nc.dram_tensor("b1r", [1, 128], F32, kind="ExternalInput")
    if has_b2:
        b2_d = nc.dram_tensor("b2r", [1, 128], F32, kind="ExternalInput")
    out_d = nc.dram_tensor("out", [G, OUT], F32, kind="ExternalOutput")
    if DEBUG_H1:
        h1dump_d = nc.dram_tensor("h1dump", [NPC, 128], FP16,
                                  kind="ExternalOutput")

    relu = mybir.ActivationFunctionType.Relu
    copyf = mybir.ActivationFunctionType.Copy

    with tile.TileContext(nc) as tc:
        nc.gpsimd.load_library(mlp)
        with ExitStack() as ctx:
            const = ctx.enter_context(tc.tile_pool(name="const", bufs=1))
            dram = ctx.enter_context(tc.tile_pool(name="dram", bufs=1,
                                                  space="DRAM"))
            mtp = ctx.enter_context(tc.tile_pool(name="mt", bufs=2))
            msgp = ctx.enter_context(tc.tile_pool(name="msg", bufs=2))
            ohp = ctx.enter_context(tc.tile_pool(name="oh", bufs=4))
            asbp = ctx.enter_context(tc.tile_pool(name="asb", bufs=2))
            rlp = ctx.enter_context(tc.tile_pool(name="rl", bufs=2))
            stp = ctx.enter_context(tc.tile_pool(name="st", bufs=2))
            ownp = ctx.enter_context(tc.tile_pool(name="own", bufs=3))
            pwp = ctx.enter_context(tc.tile_pool(name="pw", bufs=3))
            aggp = ctx.enter_context(tc.tile_pool(name="agg", bufs=2,
                                                  space="PSUM"))
            p2p = ctx.enter_context(tc.tile_pool(name="p2", bufs=1,
                                                 space="PSUM"))
            mtrp = ctx.enter_context(tc.tile_pool(name="mtr", bufs=2,
                                                  space="PSUM"))
            trp = ctx.enter_context(tc.tile_pool(name="tr", bufs=2,
                                                 space="PSUM"))
            plp = ctx.enter_context(tc.tile_pool(name="pl", bufs=1,
                                                 space="PSUM"))

            # ---- DRAM scratch + collectives --------------------------------
            xloc = dram.tile([NPC, 128], FP16)
            tabx = dram.tile([N, 128], FP16)
            h1loc = dram.tile([NPC, 128], FP16)
            tabh = dram.tile([N, 128], FP16)

            nc.sync.dma_start(xloc[:], xsh_d[:])
            nc.gpsimd.collective_compute(
                "AllGather", mybir.AluOpType.bypass,
                replica_groups=[list(range(NC))],
                ins=[xloc.opt()], outs=[tabx.opt()])

            # ---- constants -------------------------------------------------
            idx_t = const.tile([128, TOTCH * 8], I16)
            for r in range(8):
                nc.sync.dma_start(idx_t[r * 16:(r + 1) * 16, :], idx_d[:])
            dstl_t = const.tile([128, TOTCH], F32)
            nc.sync.dma_start(dstl_t[:], dstl_d[:])
            iota_t = const.tile([128, WT * 128], FP16)
            nc.sync.dma_start(iota_t[:], iota_d[:])
            iotag_t = const.tile([128, G], FP16)
            nc.sync.dma_start(iotag_t[:], iotag_d[:])
            ident_t = const.tile([128, 128], F32)
            nc.sync.dma_start(ident_t[:], ident_d[:])
            identh_t = const.tile([128, 128], FP16)
            nc.any.tensor_copy(identh_t[:], ident_t[:])
            zc_t = const.tile([1, 512], FP16)
            nc.vector.memset(zc_t[:], 0.0)
            w1_t = const.tile([4, 128], F32)
            nc.sync.dma_start(w1_t[:], w1_d[:])
            w2_t = const.tile([128, 128], F32)
            nc.sync.dma_start(w2_t[:], w2_d[:])
            wl_t = const.tile([128, OUT], F32)
            nc.sync.dma_start(wl_t[:], wl_d[:])
            ivd_t = const.tile([128, NT], F32)
            nc.sync.dma_start(ivd_t[:], ivd_d[:])
            pbat_t = const.tile([128, NT], F32)
            nc.sync.dma_start(pbat_t[:], pbat_d[:])
            pwv_t = const.tile([128, NT], F32)
            nc.sync.dma_start(pwv_t[:], pwv_d[:])
            if has_b1 or has_b2:
                sqd_t = const.tile([1, NPAD], F32)
                nc.sync.dma_start(sqd_t[:], sqd_d[:])
            if has_b1:
                b1_t = const.tile([1, 128], F32)
                nc.sync.dma_start(b1_t[:], b1_d[:])
            if has_b2:
                b2_t = const.tile([1, 128], F32)
                nc.sync.dma_start(b2_t[:], b2_d[:])

            pooled_ps = plp.tile([128, G], F32)

            CALL_CHUNKS = 48

            def layer(lnum):
                KIN = 4 if lnum == 1 else 128
                tab = tabx if lnum == 1 else tabh
                own_src = xsh_d if lnum == 1 else h1loc
                has_b = has_b1 if lnum == 1 else has_b2
                brow = (b1_t if lnum == 1 else b2_t) if has_b else None
                w_t = w1_t if lnum == 1 else w2_t
                gq = 0
                for w, wtiles in enumerate(waves):
                    msg = msgp.tile([128, CW, KIN if TGATHER else 128],
                                    FP16, tag="msg")
                    mts = {}
                    for s in range(NSC):
                        icol0, nidx, mcol0, nch = gmeta[w][s]
                        if nidx == 0:
                            continue
                        r0 = s * cfg.SC_SIZE
                        r1 = min(N, r0 + cfg.SC_SIZE)
                        for cb in range(0, nch, CALL_CHUNKS):
                            ce = min(cb + CALL_CHUNKS, nch)
                            ni = (ce - cb) * 128
                            if TGATHER:
                                mt = mtp.tile([128, ni], FP16, tag="mt")
                                nc.gpsimd.dma_gather(
                                    mt[:].rearrange("p (c i) -> p c i", c=1),
                                    tab[r0:r1, :],
                                    idx_t[:, icol0 + cb * 8:
                                          icol0 + cb * 8 + ni // 16],
                                    ni, ni, 128,
                                    transpose=True,
                                    single_packet=False,
                                    queue_num=gq % 4,
                                )
                                for k in range(ce - cb):
                                    mts[mcol0 + cb + k] = (mt, k)
                            else:
                                nc.gpsimd.dma_gather(
                                    msg[:, mcol0 + cb:mcol0 + ce, :],
                                    tab[r0:r1, :],
                                    idx_t[:, icol0 + cb * 8:
                                          icol0 + cb * 8 + ni // 16],
                                    ni, ni, 128,
                                    single_packet=False,
                                    queue_num=gq % 4,
                                )
                            gq += 1
                    # per-chunk PE transpose back to edge-major + PSUM->SBUF
                    for mcol, (mt, k) in mts.items():
                        mtr = mtrp.tile([128, 128], FP16, tag="mtr")
                        nc.tensor.transpose(
                            mtr[:], mt[:, k * 128:(k + 1) * 128], identh_t[:])
                        nc.scalar.activation(msg[:, mcol, :], mtr[:, 0:KIN],
                                             copyf)
                    # psum scatter-add accumulation
                    mms = wmms[w]
                    aggs = [aggp.tile([KIN, 512], F32, tag="agg",
                                      name=f"agg{lnum}_w{w}_{h}")
                            for h in range((len(wtiles) + 3) // 4)]
                    for agg in aggs:
                        nc.tensor.matmul(agg[:], zc_t[0:1, 0:KIN],
                                         zc_t[0:1, 0:512],
                                         start=True, stop=False,
                                         skip_group_check=True)
                    for mcol, j0, wid, gcol in mms:
                        oh = ohp.tile([128, wid * 128], FP16, tag="oh")
                        nc.vector.tensor_scalar(
                            oh[:], iota_t[:, j0 * 128:(j0 + wid) * 128],
                            dstl_t[:, gcol:gcol + 1], None,
                            mybir.AluOpType.is_equal)
                        agg = aggs[j0 // 4]
                        psl = agg[:, (j0 % 4) * 128:(j0 % 4 + wid) * 128]
                        nc.tensor.matmul(
                            psl, msg[:, mcol, 0:KIN], oh[:],
                            start=False, stop=False, skip_group_check=True)
                    # self-loop term
                    for j, t in enumerate(wtiles):
                        rows = min(128, NPC - t * 128)
                        own_t = ownp.tile([128, 128], FP16, tag="own")
                        nc.sync.dma_start(own_t[0:rows, :],
                                          own_src[t * 128:t * 128 + rows, :])
                        psl = aggs[j // 4][:, (j % 4) * 128:(j % 4) * 128 + 128]
                        nc.tensor.matmul(
                            psl, own_t[0:rows, 0:KIN], identh_t[0:rows, :],
                            start=False,
                            stop=(j % 4 == 3 or j == len(wtiles) - 1),
                            skip_group_check=True)
                    # epilogue per tile
                    for j, t in enumerate(wtiles):
                        psl = aggs[j // 4][:, (j % 4) * 128:(j % 4) * 128 + 128]
                        agg_sb = asbp.tile([KIN, 128], F32, tag="asb")
                        nc.scalar.activation(agg_sb[:], psl, copyf)
                        p2 = p2p.tile([128, 128], F32, tag="p2")
                        nc.tensor.matmul(p2[:], w_t[:], agg_sb[:],
                                         start=True, stop=not has_b)
                        if has_b:
                            nc.tensor.matmul(p2[:], brow[:],
                                             sqd_t[0:1, t * 128:t * 128 + 128],
                                             start=False, stop=True)
                        relu_sb = rlp.tile([128, 128], F32, tag="rl")
                        nc.scalar.activation(relu_sb[:], p2[:], relu)
                        tnm = trp.tile([128, 128], F32, tag="tr")
                        nc.tensor.transpose(tnm[:], relu_sb[:], ident_t[:])
                        if lnum == 1:
                            if j == 0:
                                stage = stp.tile([128, WT * 128], FP16,
                                                 tag="stage")
                            nc.scalar.activation(
                                stage[:, j * 128:j * 128 + 128], tnm[:],
                                copyf, scale=ivd_t[:, t:t + 1])
                        else:
                            tnm_sb = stp.tile([128, 128], F32, tag="tnm")
                            nc.scalar.activation(tnm_sb[:], tnm[:], copyf)
                            pw_t = pwp.tile([128, G], F32, tag="pw")
                            nc.vector.tensor_scalar(
                                pw_t[:], iotag_t[:],
                                pbat_t[:, t:t + 1], None,
                                mybir.AluOpType.is_equal)
                            nc.vector.tensor_scalar(
                                pw_t[:], pw_t[:],
                                pwv_t[:, t:t + 1], None,
                                mybir.AluOpType.mult)
                            nc.tensor.matmul(pooled_ps[:], tnm_sb[:], pw_t[:],
                                             start=(t == 0), stop=(t == NT - 1),
                                             skip_group_check=True)
                    if lnum == 1:
                        # store wave's node-major rows (fp16) into h1loc
                        base = wtiles[0] * 128
                        nfull = sum(1 for t in wtiles if (t + 1) * 128 <= NPC)
                        if nfull:
                            dst_ap = h1loc[base:base + nfull * 128, :].rearrange(
                                "(j p) f -> p j f", p=128)
                            nc.sync.dma_start(
                                dst_ap, stage[:, 0:nfull * 128]
                                .rearrange("p (j f) -> p j f", f=128))
                        for j, t in enumerate(wtiles):
                            if (t + 1) * 128 <= NPC:
                                continue
                            rows = NPC - t * 128
                            if rows > 0:
                                nc.sync.dma_start(
                                    h1loc[t * 128:t * 128 + rows, :],
                                    stage[0:rows, j * 128:(j + 1) * 128])

            layer(1)
            if DEBUG_H1:
                nc.sync.dma_start(h1dump_d[:], h1loc[:])
            nc.gpsimd.collective_compute(
                "AllGather", mybir.AluOpType.bypass,
                replica_groups=[list(range(NC))],
                ins=[h1loc.opt()], outs=[tabh.opt()])
            layer(2)

            pooled_sb = const.tile([128, G], F32)
            nc.any.tensor_copy(pooled_sb[:], pooled_ps[:])
            outp = p2p.tile([128, 128], F32, tag="p2")
            nc.tensor.matmul(outp[0:G, 0:OUT], pooled_sb[:], wl_t[:],
                             start=True, stop=True, skip_group_check=True)
            out_sb = const.tile([G, OUT], F32)
            nc.any.tensor_copy(out_sb[:], outp[0:G, 0:OUT])
            nc.sync.dma_start(out_d[:], out_sb[:])

    nc.compile()
    return nc


# --------------------------------------------------------------------------
# driver
# --------------------------------------------------------------------------

def make_in_maps(cfg, meta, W1, b1, W2, b2, Wl, bl):
    NC, NPC = cfg.NCORES, cfg.NPC
    has_b1 = bool(np.any(np.asarray(b1)))
    has_b2 = bool(np.any(np.asarray(b2)))
    assert cfg.DIN <= 4
    W1p = np.zeros((4, 128), dtype=np.float32)
    W1p[:cfg.DIN] = np.asarray(W1, dtype=np.float32)
    in_maps = []
    for c in range(NC):
        m = dict(
            xsh=meta["x_pad"][c * NPC:(c + 1) * NPC],
            idx=meta["idx16"][c],
            dstl=meta["dstl_wrap"][c],
            iota=meta["iota"], iotag=meta["iotag"], ident=meta["ident"],
            w1=W1p, w2=np.asarray(W2, np.float32),
            wl=np.asarray(Wl, np.float32),
            ivd=meta["invdeg_col"][c],
            pbat=meta["pbat_col"][c], pwv=meta["pwv_col"][c],
        )
        if has_b1 or has_b2:
            m["sqd"] = meta["sqrtdeg_row"][c]
        if has_b1:
            m["b1r"] = np.asarray(b1, np.float32).reshape(1, 128)
        if has_b2:
            m["b2r"] = np.asarray(b2, np.float32).reshape(1, 128)
        in_maps.append(m)
    return in_maps, has_b1, has_b2


def _run(cfg, meta, W1, b1, W2, b2, Wl, bl, runner):
    in_maps, has_b1, has_b2 = make_in_maps(cfg, meta, W1, b1, W2, b2, Wl, bl)
    nc = build_fused(cfg, meta, has_b1, has_b2)
    res = runner(nc, in_maps)
    total = np.sum([res[c]["out"] for c in range(cfg.NCORES)], axis=0)
    return (total + np.asarray(bl, np.float32)[None, :]).astype(np.float32)


def _hw_runner(nc, in_maps):
    # one bounded retry: the axon/fake_nrt path occasionally throws a
    # transient "mesh desynced" on the first execute after device churn
    try:
        res = run_bass_kernel_spmd(nc, in_maps,
                                   core_ids=list(range(len(in_maps))))
    except Exception:
        import time as _time
        _time.sleep(10)
        res = run_bass_kernel_spmd(nc, in_maps,
                                   core_ids=list(range(len(in_maps))))
    return res.results


def kernel(x, edge_index, batch, W1, b1, W2, b2, Wl, bl):
    cfg = FULL
    meta = preprocess(cfg, x, edge_index, batch)
    return _run(cfg, meta, W1, b1, W2, b2, Wl, bl, _hw_runner)
